# revision 1
# baseline (speedup 1.0000x reference)
"""Trainium2 Bass kernel for nn_Decoder (teacher-forced LSTM decoder).

Contract: kernel(**inputs) takes the FULL unsharded inputs (as produced by
reference.setup_inputs()) and returns the FULL [T, B, H] float32 output.

Sharding: pure data-parallel over the batch dim across 8 NeuronCores
(B=4096 -> 512 rows/core); all weights replicated; no collectives.

Per-core kernel (transposed-gates formulation):
  - table[v, :] = emb[v] @ W_ih.T + b_ih + b_hh precomputed once (bf16, DRAM)
  - per step: gatesT = W_hh @ h^T on the PE (bf16, fp32 accumulate) plus the
    input-side term fetched already-transposed via dma_gather(transpose=True)
  - sigmoid/tanh on the scalar engine, cell update on the vector engine, all
    in [H-on-partitions, B-on-free] layout so hT feeds the next step directly
  - the f32 output is reconstructed through a bf16 DRAM scratch + xbar DMA
    transpose + casting DMA, overlapped one step behind the recurrence
"""

import os
import sys

for _p in ("/opt/trn_rl_repo", os.path.expanduser("~/.axon_site/_ro/trn_rl_repo")):
    if os.path.isdir(_p) and _p not in sys.path:
        sys.path.insert(0, _p)

from contextlib import ExitStack

import numpy as np

import concourse.bass as bass
import concourse.tile as tile
from concourse import bacc, mybir
from concourse.bass_utils import run_bass_kernel_spmd
from concourse.masks import make_identity

FP32 = mybir.dt.float32
BF16 = mybir.dt.bfloat16
I32 = mybir.dt.int32
I16 = mybir.dt.int16
AF = mybir.ActivationFunctionType
P = 128
SOS = 1

N_CORES = 8
# Problem shape (hardcoded per contest contract)
B_FULL, T_STEPS, E_DIM, H_DIM, V_DIM = 4096, 20, 256, 1024, 1000


def _build(B, T, E, H, V):
    G = 4 * H
    KH = H // P            # contraction tiles over H (= h-dim chunks)
    KE = E // P
    NG = G // 512
    GC = 4 * KH            # 128-row gate chunks of gatesT
    VP = ((V + P - 1) // P) * P
    VT = VP // P
    B16 = B // 16
    GN = min(B, 256)       # tokens per (transposed) gather
    NHALF = B // GN
    GW = GN // 16          # index slots per gather
    BT = B // P
    assert B % P == 0 and H % 512 == 0 and E % P == 0

    nc = bacc.Bacc("TRN2", target_bir_lowering=False, debug=False, num_devices=1)
    labels = nc.dram_tensor("labels_batch", [B, T], I32, kind="ExternalInput").ap()
    dh = nc.dram_tensor("decoder_hidden", [B, H], FP32, kind="ExternalInput").ap()
    emb = nc.dram_tensor("emb", [V, E], FP32, kind="ExternalInput").ap()
    W_ih = nc.dram_tensor("W_ih", [G, E], FP32, kind="ExternalInput").ap()
    W_hh = nc.dram_tensor("W_hh", [G, H], FP32, kind="ExternalInput").ap()
    b_ih = nc.dram_tensor("b_ih", [G], FP32, kind="ExternalInput").ap()
    b_hh = nc.dram_tensor("b_hh", [G], FP32, kind="ExternalInput").ap()
    hidden = nc.dram_tensor("hidden", [T, B, H], FP32, kind="ExternalOutput").ap()

    w_hh_bf = nc.dram_tensor("w_hh_bf", [G, H], BF16, kind="Internal").ap()
    w_ih_bf = nc.dram_tensor("w_ih_bf", [G, E], BF16, kind="Internal").ap()
    bias_dram = nc.dram_tensor("bias_dram", [G], BF16, kind="Internal").ap()
    sos_row = nc.dram_tensor("sos_row", [G], BF16, kind="Internal").ap()
    table = nc.dram_tensor("table", [VP, G], BF16, kind="Internal").ap()
    hiddenT = nc.dram_tensor("hiddenT", [T, H, B], BF16, kind="Internal").ap()

    n_order = [n for n in range(0, NG, 2)] + [n for n in range(1, NG, 2)]

    with tile.TileContext(nc) as tc, ExitStack() as ctx:
        small = ctx.enter_context(tc.tile_pool(name="small", bufs=1))
        wpool = ctx.enter_context(tc.tile_pool(name="w", bufs=1))
        hT_pool = ctx.enter_context(tc.tile_pool(name="hT", bufs=2))

        idx16 = small.tile([P, T, B16], I16, tag="idx16")
        sosT = small.tile([P, GC], FP32, tag="sosT")
        ident = small.tile([P, P], FP32, tag="ident")
        ident_bf = small.tile([P, P], BF16, tag="ident_bf")
        w_hhT = wpool.tile([P, KH, G], BF16, tag="w_hhT")

        make_identity(nc, ident)
        make_identity(nc, ident_bf)

        with tc.tile_pool(name="prolog", bufs=1) as prolog, \
             tc.tile_pool(name="prolog2", bufs=2) as prolog2, \
             tc.tile_pool(name="pps", bufs=2, space="PSUM") as pps:
            dh_sb = []
            for m in range(BT):
                dh_m = prolog2.tile([P, H], FP32, tag="dh", name=f"dh_{m}")
                nc.sync.dma_start(dh_m, dh[m * P:(m + 1) * P, :])
                dh_sb.append(dh_m)
            emb_sb = prolog.tile([P, VT, E], FP32, tag="emb")
            full = V // P
            if V % P:
                nc.vector.memset(emb_sb[:, VT - 1, :], 0.0)
            nc.sync.dma_start(emb_sb[:, :full, :],
                              emb[:full * P].rearrange("(v p) e -> p v e", p=P))
            if V % P:
                nc.sync.dma_start(emb_sb[:V % P, full, :], emb[full * P:])
            brow_i = prolog.tile([P, G // P], FP32, tag="brow_i")
            brow_h = prolog.tile([P, G // P], FP32, tag="brow_h")
            nc.sync.dma_start(brow_i, b_ih.rearrange("(p x) -> p x", p=P))
            nc.sync.dma_start(brow_h, b_hh.rearrange("(p x) -> p x", p=P))
            lab32 = prolog2.tile([P, B16, T], I32, tag="lab32")
            labv = labels.rearrange("(bh p) t -> p bh t", p=16)
            for r in range(8):
                nc.sync.dma_start(lab32[16 * r:16 * (r + 1), :, :], labv)

            # W_ih cast: load (ACT ring) -> DVE copy -> store (SWDGE ring)
            for c in range(G // 1024):
                ws = prolog.tile([P, 8, E], FP32, tag="wsi", name=f"wsi_{c}")
                nc.scalar.dma_start(
                    ws, W_ih[c * 1024:(c + 1) * 1024].rearrange("(o p) e -> p o e", p=P))
                wb = prolog2.tile([P, 8, E], BF16, tag="wbi", name=f"wbi_{c}")
                nc.vector.tensor_copy(wb, ws)
                nc.gpsimd.dma_start(
                    w_ih_bf[c * 1024:(c + 1) * 1024].rearrange("(o p) e -> p o e", p=P), wb)
            w_ihT = prolog.tile([P, KE, G], BF16, tag="w_ihT")
            for k in range(KE):
                nc.sync.dma_start_transpose(w_ihT[:, k, :], w_ih_bf[:, k * P:(k + 1) * P])

            # W_hh cast + xbar transpose, 512-row blocks, first-needed first
            for n in n_order:
                eng = nc.sync if n % 2 == 0 else nc.scalar
                ws = prolog.tile([P, 4, H], FP32, tag="wsh", name=f"wsh_{n}")
                eng.dma_start(
                    ws, W_hh[n * 512:(n + 1) * 512].rearrange("(o p) h -> p o h", p=P))
                wb = prolog2.tile([P, 4, H], BF16, tag="wbh", name=f"wbh_{n}")
                nc.vector.tensor_copy(wb, ws)
                nc.gpsimd.dma_start(
                    w_hh_bf[n * 512:(n + 1) * 512].rearrange("(o p) h -> p o h", p=P), wb)
                for k in range(KH):
                    eng.dma_start_transpose(
                        w_hhT[:, k, n * 512:(n + 1) * 512],
                        w_hh_bf[n * 512:(n + 1) * 512, k * P:(k + 1) * P])

            v16 = lab32.bitcast(I16).rearrange("p b (t two) -> p t b two", two=2)[:, :, :, 0]
            nc.vector.tensor_copy(idx16, v16)

            brow16 = prolog.tile([P, G // P], BF16, tag="brow16")
            nc.vector.tensor_add(brow16, brow_i, brow_h)
            nc.sync.dma_start(bias_dram.rearrange("(p x) -> p x", p=P), brow16)
            bias = prolog.tile([P, G], BF16, tag="bias")
            nc.scalar.dma_start(bias, bias_dram.partition_broadcast(P))

            # h0 -> hT0 on the PE
            hT_prev = hT_pool.tile([P, KH, B], BF16, tag="hT", name="hT0")
            hT0v = hT_prev.rearrange("p k (m r) -> p k m r", r=P)
            for m in range(BT):
                trp = pps.tile([P, H], FP32, tag="tr", name=f"h0t_{m}")
                for k in range(KH):
                    nc.tensor.transpose(trp[:, k * P:(k + 1) * P],
                                        dh_sb[m][:, k * P:(k + 1) * P], ident)
                nc.vector.tensor_copy(hT0v[:, :, m, :],
                                      trp.rearrange("p (k r) -> p k r", r=P))

            # emb -> embT on the PE
            embT = prolog.tile([P, KE, VP], BF16, tag="embT")
            for e in range(KE):
                trp = pps.tile([P, VP], FP32, tag="tr", name=f"trp_{e}")
                for vt in range(VT):
                    nc.tensor.transpose(trp[:, vt * P:(vt + 1) * P],
                                        emb_sb[:, vt, e * P:(e + 1) * P], ident)
                nc.vector.tensor_copy(embT[:, e, :], trp)

            # table[v] = emb[v] @ W_ih.T + bias
            for m in range(VT):
                trow = prolog2.tile([P, G], BF16, tag="trow", name=f"trow_{m}")
                for n in range(NG):
                    ps = pps.tile([P, 512], FP32, tag="g2", name=f"tps_{m}_{n}")
                    for e in range(KE):
                        nc.tensor.matmul(ps, embT[:, e, m * P:(m + 1) * P],
                                         w_ihT[:, e, n * 512:(n + 1) * 512],
                                         start=(e == 0), stop=(e == KE - 1))
                    nc.vector.tensor_add(trow[:, n * 512:(n + 1) * 512], ps,
                                         bias[:, n * 512:(n + 1) * 512])
                if m == SOS // P:
                    # sosT[p, gc] = table[SOS, gc*P+p]: step-0 per-partition scalars
                    nc.sync.dma_start(sos_row[None, :], trow[SOS % P:SOS % P + 1, :])
                    nc.gpsimd.dma_start(sosT, sos_row.rearrange("(gc p) -> p gc", p=P))
                nc.scalar.dma_start(table[m * P:(m + 1) * P, :], trow)

        # main-loop pools (prologue SBUF/PSUM freed)
        state = ctx.enter_context(tc.tile_pool(name="state", bufs=1))
        cT = state.tile([P, KH, B], FP32, tag="cT")
        gt_pool = ctx.enter_context(tc.tile_pool(name="gt", bufs=2 * NHALF))
        act_pool = ctx.enter_context(tc.tile_pool(name="act", bufs=16))
        tmp_pool = ctx.enter_context(tc.tile_pool(name="tmp", bufs=2))
        hn_pool = ctx.enter_context(tc.tile_pool(name="hn", bufs=4))
        psum = ctx.enter_context(tc.tile_pool(name="ps", bufs=8, space="PSUM"))

        nc.vector.memset(cT, 0.0)

        def do_gathers(t):
            gts = []
            for h in range(NHALF):
                g = gt_pool.tile([P, GC, GN], BF16, tag="gt", name=f"gt_{t}_{h}")
                idxs = idx16[:, t - 1, h * GW:(h + 1) * GW]
                nc.gpsimd.dma_gather(g, table, idxs, num_idxs=GN,
                                     num_idxs_reg=GN, elem_size=G, transpose=True)
                gts.append(g)
            return gts

        def emit_output(t):
            for m in range(BT):
                hn = hn_pool.tile([P, H], BF16, tag="hn", name=f"hn_{t}_{m}")
                nc.sync.dma_start_transpose(hn, hiddenT[t][:, m * P:(m + 1) * P])
                nc.gpsimd.dma_start(hidden[t, m * P:(m + 1) * P, :], hn)

        hTd = hiddenT.rearrange("t (k p) b -> t p k b", p=P)
        gts = None
        for t in range(T):
            hT_new = hT_pool.tile([P, KH, B], BF16, tag="hT", name=f"hT_{t + 1}")
            for hc in range(KH):
                sig = {}
                for gate in range(4):
                    gc = gate * KH + hc
                    ps = psum.tile([P, B], FP32, tag="g", name=f"ps_{t}_{gc}")
                    for k in range(KH):
                        nc.tensor.matmul(ps, w_hhT[:, k, gc * P:(gc + 1) * P],
                                         hT_prev[:, k, :],
                                         start=(k == 0), stop=(k == KH - 1))
                    if t == 0:
                        nc.vector.tensor_scalar_add(ps, ps, sosT[:, gc:gc + 1])
                    else:
                        for h in range(NHALF):
                            nc.vector.tensor_add(ps[:, h * GN:(h + 1) * GN],
                                                 ps[:, h * GN:(h + 1) * GN],
                                                 gts[h][:, gc, :])
                    a = act_pool.tile([P, B], BF16, tag="act", name=f"act_{t}_{gc}")
                    nc.scalar.activation(a, ps, AF.Tanh if gate == 2 else AF.Sigmoid)
                    sig[gate] = a
                cs = cT[:, hc, :]
                tmp = tmp_pool.tile([P, B], FP32, tag="tmp", name=f"tmp_{t}_{hc}")
                nc.vector.tensor_mul(tmp, sig[0], sig[2])
                nc.vector.tensor_mul(cs, sig[1], cs)
                nc.vector.tensor_add(cs, cs, tmp)
                tca = act_pool.tile([P, B], BF16, tag="act", name=f"tc_{t}_{hc}")
                nc.scalar.activation(tca, cs, AF.Tanh)
                nc.vector.tensor_mul(hT_new[:, hc, :], sig[3], tca)
            if t < T - 1:
                nc.sync.dma_start(hTd[t], hT_new)
                if t > 0:
                    emit_output(t - 1)
                gts = do_gathers(t + 1)
            else:
                emit_output(t - 1)
                # last step: PE-transpose hT back to natural layout directly
                kper = min(KH, (2 * B) // P)
                for m in range(BT):
                    hnat = hn_pool.tile([P, H], FP32, tag="hnat", name=f"hnat_{m}")
                    for kk in range(0, KH, kper):
                        khi = min(kk + kper, KH)
                        ps = psum.tile([P, B], FP32, tag="g", name=f"lt_{m}_{kk}")
                        psb = ps.bitcast(BF16)
                        for k in range(kk, khi):
                            nc.tensor.transpose(psb[:, (k - kk) * P:(k - kk + 1) * P],
                                                hT_new[:, k, m * P:(m + 1) * P], ident_bf)
                        nc.vector.tensor_copy(hnat[:, kk * P:khi * P],
                                              psb[:, :(khi - kk) * P])
                    nc.sync.dma_start(hidden[t, m * P:(m + 1) * P, :], hnat)
            hT_prev = hT_new

    nc.compile()
    return nc


_NC_CACHE = {}


def _get_nc():
    key = (B_FULL, T_STEPS, E_DIM, H_DIM, V_DIM)
    if key not in _NC_CACHE:
        _NC_CACHE[key] = _build(B_FULL // N_CORES, T_STEPS, E_DIM, H_DIM, V_DIM)
    return _NC_CACHE[key]


def kernel(labels_batch, decoder_hidden, emb, W_ih, W_hh, b_ih, b_hh):
    labels_batch = np.ascontiguousarray(np.asarray(labels_batch, dtype=np.int32))
    decoder_hidden = np.ascontiguousarray(np.asarray(decoder_hidden, dtype=np.float32))
    emb = np.ascontiguousarray(np.asarray(emb, dtype=np.float32))
    W_ih = np.ascontiguousarray(np.asarray(W_ih, dtype=np.float32))
    W_hh = np.ascontiguousarray(np.asarray(W_hh, dtype=np.float32))
    b_ih = np.ascontiguousarray(np.asarray(b_ih, dtype=np.float32))
    b_hh = np.ascontiguousarray(np.asarray(b_hh, dtype=np.float32))

    B = B_FULL // N_CORES
    nc = _get_nc()
    in_maps = [{
        "labels_batch": np.ascontiguousarray(labels_batch[c * B:(c + 1) * B]),
        "decoder_hidden": np.ascontiguousarray(decoder_hidden[c * B:(c + 1) * B]),
        "emb": emb,
        "W_ih": W_ih,
        "W_hh": W_hh,
        "b_ih": b_ih,
        "b_hh": b_hh,
    } for c in range(N_CORES)]
    res = run_bass_kernel_spmd(nc, in_maps, core_ids=list(range(N_CORES)))
    return np.concatenate([res.results[c]["hidden"] for c in range(N_CORES)], axis=1)



# revision 4
# speedup vs baseline: 1.3362x; 1.3362x over previous
"""Trainium2 Bass kernel for nn_Decoder (teacher-forced LSTM decoder).

Contract: kernel(**inputs) takes the FULL unsharded inputs (as produced by
reference.setup_inputs()) and returns the FULL [T, B, H] float32 output.

Sharding: pure data-parallel over the batch dim across 8 NeuronCores
(B=4096 -> 512 rows/core); all weights replicated; no collectives.

Per-core kernel (transposed-gates formulation, fp8 DoubleRow recurrence):
  - table[v, :] = (emb[v] @ W_ih.T) * SWH precomputed once (bf16, DRAM);
    the bias rides the scalar-engine activation's per-partition bias operand
  - W_hh is cast to bf16, transposed on the PE (no DRAM round-trip), and
    stored as fp8e4 scaled by S_W
  - h is kept in bf16 (output path) and additionally as fp8e4 scaled by S_H
  - per step: gatesT = W8 @ h8^T via DoubleRow fp8 matmuls (2 contraction
    rows per pass -> 2x PE throughput); input-side term fetched via
    dma_gather(transpose=True); activation applies the 1/(S_W*S_H) descale
  - step 0 adds a second fp8 residual operand for h0 (fp8+fp8 ~ 11-bit h0)
    to tame the large-magnitude gaussian decoder_hidden
  - sigmoid/tanh on the scalar engine, cell update on the vector engine, all
    in [H-on-partitions, B-on-free] layout so hT feeds the next step directly
  - the f32 output is reconstructed through a bf16 DRAM scratch + xbar DMA
    transpose + casting DMA, overlapped one step behind the recurrence
"""

import os
import sys

for _p in ("/opt/trn_rl_repo", os.path.expanduser("~/.axon_site/_ro/trn_rl_repo")):
    if os.path.isdir(_p) and _p not in sys.path:
        sys.path.insert(0, _p)

from contextlib import ExitStack

import numpy as np

import concourse.bass as bass
import concourse.tile as tile
from concourse import bacc, mybir
from concourse.bass_utils import run_bass_kernel_spmd
from concourse.masks import make_identity

FP32 = mybir.dt.float32
BF16 = mybir.dt.bfloat16
F8 = mybir.dt.float8e4
I32 = mybir.dt.int32
I16 = mybir.dt.int16
AF = mybir.ActivationFunctionType
DR = mybir.MatmulPerfMode.DoubleRow
P = 128
SOS = 1

S_W = 4096.0            # W_hh fp8 scale (|W|<=1/32 -> <=128, under TRN e4m3 max 240)
S_H = 32.0              # h fp8 scale (|h|<1 after step 0; |h0|<7 from N(0,1))
SWH = S_W * S_H         # folded into the table; activation descales

N_CORES = 8
# Problem shape (hardcoded per contest contract)
B_FULL, T_STEPS, E_DIM, H_DIM, V_DIM = 4096, 20, 256, 1024, 1000


def _build(B, T, E, H, V):
    G = 4 * H
    KH = H // P            # contraction tiles over H (= h-dim chunks)
    KK = KH // 2           # DoubleRow contraction pair-tiles
    KE = E // P
    NG = G // 512
    GC = 4 * KH            # 128-row gate chunks of gatesT
    VP = ((V + P - 1) // P) * P
    VT = VP // P
    B16 = B // 16
    GN = min(B, 256)       # tokens per (transposed) gather
    NHALF = B // GN
    GW = GN // 16          # index slots per gather
    BT = B // P
    assert B % P == 0 and H % 512 == 0 and E % P == 0 and KH % 2 == 0

    nc = bacc.Bacc("TRN2", target_bir_lowering=False, debug=False, num_devices=1)
    labels = nc.dram_tensor("labels_batch", [B, T], I32, kind="ExternalInput").ap()
    dh = nc.dram_tensor("decoder_hidden", [B, H], FP32, kind="ExternalInput").ap()
    emb = nc.dram_tensor("emb", [V, E], FP32, kind="ExternalInput").ap()
    W_ih = nc.dram_tensor("W_ih", [G, E], FP32, kind="ExternalInput").ap()
    W_hh = nc.dram_tensor("W_hh", [G, H], FP32, kind="ExternalInput").ap()
    b_ih = nc.dram_tensor("b_ih", [G], FP32, kind="ExternalInput").ap()
    b_hh = nc.dram_tensor("b_hh", [G], FP32, kind="ExternalInput").ap()
    hidden = nc.dram_tensor("hidden", [T, B, H], FP32, kind="ExternalOutput").ap()

    sos_row = nc.dram_tensor("sos_row", [G], BF16, kind="Internal").ap()
    table = nc.dram_tensor("table", [VP, G], BF16, kind="Internal").ap()
    hiddenT = nc.dram_tensor("hiddenT", [T, H, B], BF16, kind="Internal").ap()

    n_order = [n for n in range(0, NG, 2)] + [n for n in range(1, NG, 2)]

    with tile.TileContext(nc) as tc, ExitStack() as ctx:
        small = ctx.enter_context(tc.tile_pool(name="small", bufs=1))
        wpool = ctx.enter_context(tc.tile_pool(name="w", bufs=1))
        hT_pool = ctx.enter_context(tc.tile_pool(name="hT", bufs=2))
        h8_pool = ctx.enter_context(tc.tile_pool(name="h8", bufs=2))

        idx16 = small.tile([P, T, B16], I16, tag="idx16")
        sosT = small.tile([P, GC], FP32, tag="sosT")
        sosb = small.tile([P, GC], FP32, tag="sosb")
        biasT = small.tile([P, GC], FP32, tag="biasT")
        ident = small.tile([P, P], FP32, tag="ident")
        ident_bf = small.tile([P, P], BF16, tag="ident_bf")
        w8T = wpool.tile([P, KH, G], F8, tag="w8T")
        h8r0 = wpool.tile([P, KH, B], F8, tag="h8r0")

        make_identity(nc, ident)
        make_identity(nc, ident_bf)

        h8_prev = h8_pool.tile([P, KH, B], F8, tag="h8", name="h8_0")

        with tc.tile_pool(name="prolog", bufs=1) as prolog, \
             tc.tile_pool(name="prolog2", bufs=2) as prolog2, \
             tc.tile_pool(name="pps", bufs=2, space="PSUM") as pps:
            # --- input loads ---
            dh_sb = []
            for m in range(BT):
                dh_m = prolog.tile([P, H], FP32, tag=f"dh{m}")
                nc.sync.dma_start(dh_m, dh[m * P:(m + 1) * P, :])
                dh_sb.append(dh_m)
            emb_sb = prolog.tile([P, VT, E], FP32, tag="emb")
            full = V // P
            if V % P:
                nc.vector.memset(emb_sb[:, VT - 1, :], 0.0)
            nc.sync.dma_start(emb_sb[:, :full, :],
                              emb[:full * P].rearrange("(v p) e -> p v e", p=P))
            if V % P:
                nc.sync.dma_start(emb_sb[:V % P, full, :], emb[full * P:])
            # bias in gatesT layout: biasT[p, gc] = b_ih[gc*P+p] + b_hh[gc*P+p]
            browi = prolog.tile([P, GC], FP32, tag="browi")
            browh = prolog.tile([P, GC], FP32, tag="browh")
            nc.sync.dma_start(browi, b_ih.rearrange("(x p) -> p x", p=P))
            nc.sync.dma_start(browh, b_hh.rearrange("(x p) -> p x", p=P))
            nc.vector.tensor_add(biasT, browi, browh)
            lab32 = prolog2.tile([P, B16, T], I32, tag="lab32")
            labv = labels.rearrange("(bh p) t -> p bh t", p=16)
            for r in range(8):
                nc.sync.dma_start(lab32[16 * r:16 * (r + 1), :, :], labv)

            # --- h0 -> PE transpose (fp32) -> fp8 a + fp8 residual, scale S_H ---
            h8av = h8_prev.rearrange("p k (m r) -> p k m r", r=P)
            h8rv = h8r0.rearrange("p k (m r) -> p k m r", r=P)
            for m in range(BT):
                trp = pps.tile([P, H], FP32, tag="tr", name=f"h0t_{m}")
                for k in range(KH):
                    nc.tensor.transpose(trp[:, k * P:(k + 1) * P],
                                        dh_sb[m][:, k * P:(k + 1) * P], ident)
                t32 = prolog2.tile([P, H], FP32, tag="t32", name=f"t32_{m}")
                nc.vector.tensor_scalar_mul(t32, trp, S_H)
                t32v = t32.rearrange("p (k r) -> p k r", r=P)
                nc.vector.tensor_copy(h8av[:, :, m, :], t32v)
                nc.vector.tensor_sub(h8rv[:, :, m, :], t32v, h8av[:, :, m, :])

            # --- W_ih: load fp32 -> gpsimd cast bf16 -> PE transpose (*SWH) ---
            wih_bf = []
            for c in range(G // 1024):
                ws = prolog2.tile([P, 8, E], FP32, tag="wsi", name=f"wsi_{c}")
                nc.scalar.dma_start(
                    ws, W_ih[c * 1024:(c + 1) * 1024].rearrange("(o p) e -> p o e", p=P))
                wb = prolog2.tile([P, 8, E], BF16, tag="wbi", name=f"wbi_{c}")
                nc.gpsimd.tensor_copy(wb, ws)
                wih_bf.append(wb)
            w_ihT = prolog.tile([P, KE, G], BF16, tag="w_ihT")
            for c in range(G // 1024):
                for o in range(8):
                    trp = pps.tile([P, 512], FP32, tag="wtr", name=f"itr_{c}_{o}")
                    trb = trp.bitcast(BF16)
                    for e in range(KE):
                        nc.tensor.transpose(trb[:, e * P:(e + 1) * P],
                                            wih_bf[c][:, o, e * P:(e + 1) * P], ident_bf)
                    nc.vector.tensor_scalar_mul(
                        w_ihT[:, :, c * 1024 + o * P:c * 1024 + o * P + P],
                        trb.rearrange("p (e r) -> p e r", r=P)[:, :KE, :], SWH)

            # --- W_hh: load fp32 (4 queues) -> ACT cast bf16 -> PE transpose
            #     -> DVE scaled cast to fp8 w8T. First-needed blocks first. ---
            qs = [nc.sync, nc.scalar, nc.gpsimd]
            for i, n in enumerate(n_order):
                eng = qs[i % 3]
                ws = prolog2.tile([P, 4, H], FP32, tag="wsh", name=f"wsh_{n}")
                eng.dma_start(
                    ws, W_hh[n * 512:(n + 1) * 512].rearrange("(o p) h -> p o h", p=P))
                wb = prolog2.tile([P, 4, H], BF16, tag="wbh", name=f"wbh_{n}")
                nc.scalar.activation(wb, ws, AF.Copy)
                for o in range(4):
                    trp = pps.tile([P, 512], FP32, tag="wtr", name=f"wtr_{n}_{o}")
                    trb = trp.bitcast(BF16)
                    for k in range(KH):
                        nc.tensor.transpose(trb[:, k * P:(k + 1) * P],
                                            wb[:, o, k * P:(k + 1) * P], ident_bf)
                    col = n * 512 + o * P
                    nc.vector.tensor_scalar_mul(
                        w8T[:, :, col:col + P],
                        trb.rearrange("p (k r) -> p k r", r=P), S_W)

            v16 = lab32.bitcast(I16).rearrange("p b (t two) -> p t b two", two=2)[:, :, :, 0]
            nc.vector.tensor_copy(idx16, v16)

            # --- emb -> embT on the PE ---
            embT = prolog.tile([P, KE, VP], BF16, tag="embT")
            for e in range(KE):
                trp = pps.tile([P, VP], FP32, tag="tr", name=f"trp_{e}")
                for vt in range(VT):
                    nc.tensor.transpose(trp[:, vt * P:(vt + 1) * P],
                                        emb_sb[:, vt, e * P:(e + 1) * P], ident)
                nc.vector.tensor_copy(embT[:, e, :], trp)

            # --- table[v] = (emb[v] @ W_ih.T) * SWH (no bias; bias rides the
            #     activation bias operand). ACT copies psum -> bf16 rows. ---
            for m in range(VT):
                trow = prolog2.tile([P, G], BF16, tag="trow", name=f"trow_{m}")
                for n in range(NG):
                    ps = pps.tile([P, 512], FP32, tag="g2", name=f"tps_{m}_{n}")
                    for e in range(KE):
                        nc.tensor.matmul(ps, embT[:, e, m * P:(m + 1) * P],
                                         w_ihT[:, e, n * 512:(n + 1) * 512],
                                         start=(e == 0), stop=(e == KE - 1))
                    nc.scalar.activation(trow[:, n * 512:(n + 1) * 512], ps, AF.Copy)
                if m == SOS // P:
                    # sosT[p, gc] = table[SOS, gc*P+p]: step-0 per-partition scalars
                    nc.sync.dma_start(sos_row[None, :], trow[SOS % P:SOS % P + 1, :])
                    nc.gpsimd.dma_start(sosT, sos_row.rearrange("(gc p) -> p gc", p=P))
                nc.scalar.dma_start(table[m * P:(m + 1) * P, :], trow)
            # combined step-0 activation bias: sos x-term (descaled) + bias
            nc.vector.tensor_scalar_mul(sosb, sosT, 1.0 / SWH)
            nc.vector.tensor_add(sosb, sosb, biasT)

        # main-loop pools (prologue SBUF/PSUM freed)
        state = ctx.enter_context(tc.tile_pool(name="state", bufs=1))
        cT = state.tile([P, KH, B], FP32, tag="cT")
        gt_pool = ctx.enter_context(tc.tile_pool(name="gt", bufs=2 * NHALF))
        act_pool = ctx.enter_context(tc.tile_pool(name="act", bufs=16))
        tmp_pool = ctx.enter_context(tc.tile_pool(name="tmp", bufs=2))
        hn_pool = ctx.enter_context(tc.tile_pool(name="hn", bufs=4))
        psum = ctx.enter_context(tc.tile_pool(name="ps", bufs=8, space="PSUM"))

        nc.vector.memset(cT, 0.0)

        def do_gathers(t):
            gts = []
            for h in range(NHALF):
                g = gt_pool.tile([P, GC, GN], BF16, tag="gt", name=f"gt_{t}_{h}")
                idxs = idx16[:, t - 1, h * GW:(h + 1) * GW]
                nc.gpsimd.dma_gather(g, table, idxs, num_idxs=GN,
                                     num_idxs_reg=GN, elem_size=G, transpose=True)
                gts.append(g)
            return gts

        def emit_output(t):
            for m in range(BT):
                hn = hn_pool.tile([P, H], BF16, tag="hn", name=f"hn_{t}_{m}")
                nc.sync.dma_start_transpose(hn, hiddenT[t][:, m * P:(m + 1) * P])
                nc.gpsimd.dma_start(hidden[t, m * P:(m + 1) * P, :], hn)

        hTd = hiddenT.rearrange("t (k p) b -> t p k b", p=P)
        gts = None
        for t in range(T):
            hT_new = hT_pool.tile([P, KH, B], BF16, tag="hT", name=f"hT_{t + 1}")
            h8_new = h8_pool.tile([P, KH, B], F8, tag="h8", name=f"h8_{t + 1}")
            for hc in range(KH):
                sig = {}
                for gate in range(4):
                    gc = gate * KH + hc
                    ps = psum.tile([P, B], FP32, tag="g", name=f"ps_{t}_{gc}")
                    wcol = w8T[:, :, gc * P:(gc + 1) * P]
                    for kk in range(KK):
                        nc.tensor.matmul(ps, wcol[:, 2 * kk:2 * kk + 2, :],
                                         h8_prev[:, 2 * kk:2 * kk + 2, :],
                                         start=(kk == 0),
                                         stop=(kk == KK - 1 and t > 0),
                                         perf_mode=DR)
                    if t == 0:
                        for kk in range(KK):
                            nc.tensor.matmul(ps, wcol[:, 2 * kk:2 * kk + 2, :],
                                             h8r0[:, 2 * kk:2 * kk + 2, :],
                                             start=False, stop=(kk == KK - 1),
                                             perf_mode=DR)
                    else:
                        for h in range(NHALF):
                            nc.vector.tensor_add(ps[:, h * GN:(h + 1) * GN],
                                                 ps[:, h * GN:(h + 1) * GN],
                                                 gts[h][:, gc, :])
                    a = act_pool.tile([P, B], BF16, tag="act", name=f"act_{t}_{gc}")
                    bias_ap = (sosb if t == 0 else biasT)[:, gc:gc + 1]
                    nc.scalar.activation(a, ps, AF.Tanh if gate == 2 else AF.Sigmoid,
                                         scale=1.0 / SWH, bias=bias_ap)
                    sig[gate] = a
                cs = cT[:, hc, :]
                tmp = tmp_pool.tile([P, B], FP32, tag="tmp", name=f"tmp_{t}_{hc}")
                nc.vector.tensor_mul(tmp, sig[0], sig[2])
                nc.vector.tensor_mul(cs, sig[1], cs)
                nc.vector.tensor_add(cs, cs, tmp)
                tca = act_pool.tile([P, B], BF16, tag="act", name=f"tc_{t}_{hc}")
                nc.scalar.activation(tca, cs, AF.Tanh)
                nc.vector.tensor_mul(hT_new[:, hc, :], sig[3], tca)
                nc.vector.tensor_scalar_mul(h8_new[:, hc, :], hT_new[:, hc, :], S_H)
            if t < T - 1:
                nc.sync.dma_start(hTd[t], hT_new)
                if t > 0:
                    emit_output(t - 1)
                gts = do_gathers(t + 1)
            else:
                emit_output(t - 1)
                # last step: PE-transpose hT back to natural layout directly
                kper = min(KH, (2 * B) // P)
                for m in range(BT):
                    hnat = hn_pool.tile([P, H], FP32, tag="hnat", name=f"hnat_{m}")
                    for kk in range(0, KH, kper):
                        khi = min(kk + kper, KH)
                        ps = psum.tile([P, B], FP32, tag="g", name=f"lt_{m}_{kk}")
                        psb = ps.bitcast(BF16)
                        for k in range(kk, khi):
                            nc.tensor.transpose(psb[:, (k - kk) * P:(k - kk + 1) * P],
                                                hT_new[:, k, m * P:(m + 1) * P], ident_bf)
                        nc.vector.tensor_copy(hnat[:, kk * P:khi * P],
                                              psb[:, :(khi - kk) * P])
                    nc.sync.dma_start(hidden[t, m * P:(m + 1) * P, :], hnat)
            h8_prev = h8_new

    nc.compile()
    return nc


_NC_CACHE = {}


def _get_nc():
    key = (B_FULL, T_STEPS, E_DIM, H_DIM, V_DIM)
    if key not in _NC_CACHE:
        _NC_CACHE[key] = _build(B_FULL // N_CORES, T_STEPS, E_DIM, H_DIM, V_DIM)
    return _NC_CACHE[key]


def kernel(labels_batch, decoder_hidden, emb, W_ih, W_hh, b_ih, b_hh):
    labels_batch = np.ascontiguousarray(np.asarray(labels_batch, dtype=np.int32))
    decoder_hidden = np.ascontiguousarray(np.asarray(decoder_hidden, dtype=np.float32))
    emb = np.ascontiguousarray(np.asarray(emb, dtype=np.float32))
    W_ih = np.ascontiguousarray(np.asarray(W_ih, dtype=np.float32))
    W_hh = np.ascontiguousarray(np.asarray(W_hh, dtype=np.float32))
    b_ih = np.ascontiguousarray(np.asarray(b_ih, dtype=np.float32))
    b_hh = np.ascontiguousarray(np.asarray(b_hh, dtype=np.float32))

    B = B_FULL // N_CORES
    nc = _get_nc()
    in_maps = [{
        "labels_batch": np.ascontiguousarray(labels_batch[c * B:(c + 1) * B]),
        "decoder_hidden": np.ascontiguousarray(decoder_hidden[c * B:(c + 1) * B]),
        "emb": emb,
        "W_ih": W_ih,
        "W_hh": W_hh,
        "b_ih": b_ih,
        "b_hh": b_hh,
    } for c in range(N_CORES)]
    res = run_bass_kernel_spmd(nc, in_maps, core_ids=list(range(N_CORES)))
    return np.concatenate([res.results[c]["hidden"] for c in range(N_CORES)], axis=1)
